# revision 71
# baseline (speedup 1.0000x reference)
"""DeepseekV4 Mega-MoE experts layer on 8 Trainium2 NeuronCores.

Strategy (expert-parallel, per sharding hint):
  - 16 experts sharded 2-per-core across 8 cores; each core receives its two
    experts' weights (losslessly converted: mxfp4*ue8m0 dequant values are
    exactly representable in TRN fp8_e4m3 for both w13 and w2).
  - Staging fp8 quantization of hidden_states is replicated on every core
    (direct fp32->fp8e4 cast; matches the reference group-scaled round trip
    except for deep-subnormal values, rel err ~6e-4).
  - Tokens are gathered per expert on-device with a one-hot matmul (the
    "all-to-all"), expert MLP runs on the gathered subset, and the host sums
    the per-expert outputs (the "combine" all-reduce).

Device schedule (v4):
  The DMA subsystem is descriptor-bound (~4 ns/descriptor aggregate), so
  every operand is repacked host-side into a partition-major layout that
  yields large contiguous descriptors: x column blocks 8KB/row, w13 slabs
  6KB, w2 12KB, gather matrix 2KB, outputs 2KB.  All input DMAs are pushed
  in consumption order on the sync ring (gather matrix + combine weights on
  the scalar ring in parallel).

  x streams in column blocks (last two half-size to shorten the pipeline
  tail); each block is quantized to fp8, gather-transposed, and consumed by
  mm1-e0 for BOTH token tiles, so PE work per block matches DMA bytes per
  block.  mm1-e1 follows m-interleaved (m0 two steps ahead so acts-e0 has
  released its PSUM accumulator slot).  Acts (silu * comb * up) are sliced
  in halves across ACT and DVE.  Phase 3 swaps the h-accumulator PSUM pool
  for deeper transpose/mm2 rings (3 banks each), so a-transposes never
  contend with mm2 accumulation; outputs are copied per 512-col piece and
  DMA'd in 1024-col halves on alternating rings.
"""

import sys

if "/opt/trn_rl_repo" not in sys.path:
    sys.path.insert(0, "/opt/trn_rl_repo")

import numpy as np
import ml_dtypes

T, D, I, E, TOPK, GROUP = 512, 2048, 768, 16, 8, 32
N_CORES = 8
E_LOC = E // N_CORES  # experts per core

FP8 = ml_dtypes.float8_e4m3      # TRN FP8_EXP4 (max 240) == bass dt.float8e4
BF16 = ml_dtypes.bfloat16

_FP4_TABLE = np.array(
    [0.0, 0.5, 1.0, 1.5, 2.0, 3.0, 4.0, 6.0,
     -0.0, -0.5, -1.0, -1.5, -2.0, -3.0, -4.0, -6.0], dtype=np.float32)

BLK = [2, 2, 4, 4, 4]            # d-chunks (j) per x column block
NBT = 2                          # number of half (2j) blocks, placed first


def _dequant_mxfp4(w_packed, sf):
    lo = _FP4_TABLE[w_packed & 0xF]
    hi = _FP4_TABLE[(w_packed >> 4) & 0xF]
    w = np.stack([lo, hi], axis=-1).reshape(*w_packed.shape[:-1], -1)
    s = (sf.astype(np.uint32) << 23).view(np.float32)
    w = w.reshape(*sf.shape, GROUP) * s[..., None]
    return w.reshape(*w_packed.shape[:-1], 2 * w_packed.shape[-1])


_PROGRAM_CACHE = {}


def _build_program(cap, split_waits=True):
    import concourse.bass as bass
    import concourse.mybir as mybir
    import concourse.tile as tile
    from concourse.masks import make_identity

    _TC = tile.TileContext

    def _split_excess_waits(nc):
        # This walrus build accepts only ONE sem-wait per instruction; hoist
        # extra waits onto standalone EventSemaphore (pure-wait) instructions
        # on the same engine, which execute in order ahead of the original.
        n = 0
        for f in nc.m.functions:
            for b in f.blocks:
                out = []
                for ins in b.instructions:
                    si = ins.sync_info
                    waits = list(si.on_wait) if (si and si.on_wait) else []
                    if len(waits) > 1:
                        for k, w in enumerate(waits[:-1]):
                            out.append(mybir.InstEventSemaphore(
                                name=f"{ins.name}-xw{k}", engine=ins.engine,
                                ins=[], outs=[],
                                sync_info=mybir.SyncInfo(
                                    on_wait=[w], on_update=[])))
                            n += 1
                        si.on_wait = waits[-1:]
                    out.append(ins)
                b.instructions = out
        return n

    dt = mybir.dt
    MT = cap // 128                 # token tiles per expert
    SLOTS = E_LOC * cap             # gathered slots across local experts
    DT, FT, IT = D // 128, 2 * I // 512, I // 128   # 16, 3, 6
    TT = T // 128                   # 4 token chunks
    SJ = 4                          # j per w13-e1 slab
    NS = DT // SJ                   # w13-e1 slabs (4)
    DQ = D // 512                   # mm2 output column pieces (4)
    HI = I // 2                     # act half-slice width (384)

    nc = bass.Bass()
    # x ships as fp16: its 10 mantissa bits keep the fp32->fp16->fp8e4 double
    # rounding within ~8e-3 of the reference staging quantization (bf16's 7
    # bits do not), and halving x halves the phase-1 DMA-bound prefix
    xt_d = nc.dram_tensor("xt", [NBT, 128, TT * 256], dt.float16,
                          kind="ExternalInput")
    xb_d = nc.dram_tensor("xb", [len(BLK) - NBT, 128, TT * 512], dt.float16,
                          kind="ExternalInput")
    g_d = nc.dram_tensor("g", [128, TT * SLOTS], dt.float8e4, kind="ExternalInput")
    w13_d = nc.dram_tensor("w13t", [E_LOC, 128, DT * 2 * I], dt.float8e4,
                           kind="ExternalInput")
    w2_d = nc.dram_tensor("w2t", [E_LOC, 128, IT * D], dt.float8e4,
                          kind="ExternalInput")
    comb_d = nc.dram_tensor("combg", [128, E_LOC * MT], dt.float32, kind="ExternalInput")
    ye_d = nc.dram_tensor("ye", [E_LOC, cap, D], dt.bfloat16, kind="ExternalOutput")

    with _TC(nc) as tc:
        with (
            tc.tile_pool(name="const", bufs=1) as constp,
            tc.tile_pool(name="xin", bufs=3) as xinp,
            tc.tile_pool(name="x8", bufs=2) as x8p,
            tc.tile_pool(name="xg", bufs=1) as xgp,
            tc.tile_pool(name="wts", bufs=1) as wtsp,
            tc.tile_pool(name="act", bufs=2) as actp,
            tc.tile_pool(name="apool", bufs=4) as apool,
            tc.tile_pool(name="yout", bufs=3) as youtp,
            tc.tile_pool(name="ps_g", bufs=2, space="PSUM") as psg,
        ):
            # ---- DMA pushes, consumption order ----
            # gather matrix FIRST on the sync ring: the scalar ring's queue is
            # served round-robin against the sync stream, which starved it
            gmat = constp.tile([128, TT, SLOTS], dt.float8e4, tag="g")
            nc.sync.dma_start(gmat[:], g_d.rearrange("p (c s) -> p c s", s=SLOTS))
            combg = constp.tile([128, E_LOC * MT], dt.float32, tag="cg")
            nc.scalar.dma_start(combg[:], comb_d[:])

            w13r = [w13_d[e].rearrange("p (j f) -> p j f", f=2 * I)
                    for e in range(E_LOC)]
            xin = []
            w13s0 = []
            j0 = 0
            for b, nj in enumerate(BLK):
                xi = xinp.tile([128, TT, nj * 128], dt.float16, tag=f"xin{nj}",
                               name=f"xin_{b}")
                if b < NBT:
                    src = xt_d[b].rearrange("p (c f) -> p c f", f=256)
                else:
                    src = xb_d[b - NBT].rearrange("p (c f) -> p c f", f=512)
                nc.sync.dma_start(xi[:], src)
                xin.append(xi)
                wt = wtsp.tile([128, nj, 2 * I], dt.float8e4, tag=f"w13_0_{b}")
                nc.sync.dma_start(wt[:], w13r[0][:, j0:j0 + nj, :])
                w13s0.append(wt)
                j0 += nj
            w13t1 = []
            for s in range(NS):
                wt = wtsp.tile([128, SJ, 2 * I], dt.float8e4, tag=f"w13_1_{s}")
                nc.sync.dma_start(wt[:], w13r[1][:, s * SJ:(s + 1) * SJ, :])
                w13t1.append(wt)
            w2t = []
            for e in range(E_LOC):
                wt = wtsp.tile([128, IT, D], dt.float8e4, tag=f"w2_{e}")
                nc.sync.dma_start(
                    wt[:], w2_d[e].rearrange("p (k f) -> p k f", f=D))
                w2t.append(wt)

            ident = constp.tile([128, 128], dt.bfloat16)
            make_identity(nc, ident[:])

            def w13_ap(e, u):
                # [128, 2, 2I] slab slice covering d-chunks 2u, 2u+1
                if e == 1:
                    s, jj = divmod(2 * u, SJ)
                    return w13t1[s][:, jj:jj + 2, :]
                j = 2 * u
                acc = 0
                for b, nj in enumerate(BLK):
                    if j < acc + nj:
                        return w13s0[b][:, j - acc:j - acc + 2, :]
                    acc += nj
                raise IndexError(u)

            xgT = xgp.tile([128, DT, SLOTS], dt.float8e4, tag="xgT")
            h = {}

            def mm1_ops(e, m, u):
                # one DoubleRow contraction step (256 d) for all FT f-blocks
                hm = h[(e, m)]
                w = w13_ap(e, u)
                for fb in range(FT):
                    nc.tensor.matmul(
                        hm[:, fb * 512:(fb + 1) * 512],
                        xgT[:, 2 * u:2 * u + 2,
                            e * cap + m * 128:e * cap + (m + 1) * 128],
                        w[:, :, fb * 512:(fb + 1) * 512],
                        start=(u == 0), stop=(u == DT // 2 - 1),
                        perf_mode=mybir.MatmulPerfMode.DoubleRow)

            a_tiles = {}

            def acts(e, m):
                # a = (silu(gate) * comb) * up, in half-slices pipelined
                # across ACT (silu) and DVE (scale-mult)
                hm = h[(e, m)]
                cg = combg[:, e * MT + m:e * MT + m + 1]
                a = apool.tile([128, I], dt.bfloat16, tag="a", name=f"a_{e}_{m}")
                for hf in range(2):
                    sl = slice(hf * HI, (hf + 1) * HI)
                    sg = actp.tile([128, HI], dt.float32, tag=f"sg_{hf}",
                                   name=f"sg_{e}_{m}_{hf}")
                    nc.scalar.activation(
                        sg[:], hm[:, hf * HI:(hf + 1) * HI],
                        mybir.ActivationFunctionType.Silu)
                    nc.vector.scalar_tensor_tensor(
                        a[:, sl], sg[:], cg, hm[:, I + hf * HI:I + (hf + 1) * HI],
                        op0=mybir.AluOpType.mult, op1=mybir.AluOpType.mult)
                a_tiles[(e, m)] = a

            with tc.tile_pool(name="ps_h", bufs=2, space="PSUM") as psh:
                # ---- phase 1: stream x blocks -> quantize -> gather, with
                # mm1-e0 (both m tiles) interleaved per block ----
                for m in range(MT):
                    h[(0, m)] = psh.tile([128, 2 * I], dt.float32, tag="acc",
                                         name=f"h_0_{m}")
                # quantize alternates DVE/ACT; each gather-psum copy is split
                # in half across BOTH engines so the PSUM ring slot releases
                # in ~0.35us instead of 0.7us (the copies pace the gathers)
                j0 = 0
                for b, nj in enumerate(BLK):
                    x8 = x8p.tile([128, TT, nj * 128], dt.float8e4, tag=f"x8{nj}",
                                  name=f"x8_{b}")
                    for c in range(TT):
                        if c % 2 == 0:
                            nc.vector.tensor_copy(x8[:, c, :], xin[b][:, c, :])
                        else:
                            nc.scalar.copy(x8[:, c, :], xin[b][:, c, :])
                    for jj in range(nj):
                        j = j0 + jj
                        pg = psg.tile([128, SLOTS], dt.float32, tag="sm",
                                      name=f"pg_{j}")
                        for v in range(TT // 2):
                            nc.tensor.matmul(
                                pg[:],
                                x8[:, 2 * v:2 * v + 2, jj * 128:(jj + 1) * 128],
                                gmat[:, 2 * v:2 * v + 2, :],
                                start=(v == 0), stop=(v == TT // 2 - 1),
                                perf_mode=mybir.MatmulPerfMode.DoubleRow)
                        hs = SLOTS // 2
                        nc.vector.tensor_copy(xgT[:, j, 0:hs], pg[:, 0:hs])
                        nc.scalar.copy(xgT[:, j, hs:SLOTS], pg[:, hs:SLOTS])
                    for u in range(j0 // 2, (j0 + nj) // 2):
                        for m in range(MT):
                            mm1_ops(0, m, u)
                    j0 += nj

                # ---- phase 2: acts-e0, mm1-e1 (m0 head start), acts-e1 ----
                for m in range(MT):
                    acts(0, m)
                for m in range(MT):
                    h[(1, m)] = psh.tile([128, 2 * I], dt.float32, tag="acc",
                                         name=f"h_1_{m}")
                # m-serial: m0's full u-loop gives acts-e0-m1 a 5 µs window
                # to release h(1,1)'s PSUM slot; acts-e1-m0 then runs under
                # mm1-e1-m1 so its result is ready for phase 3
                for m in range(MT):
                    for u in range(DT // 2):
                        mm1_ops(1, m, u)
                    acts(1, m)

            # ---- phase 3: transposes (psg pool — no pool barrier, they
            # bridge the acts-e1 latency), then mm2 stages (ps3 pool, whose
            # allocation barrier only gates the first mm2) ----
            ceng = [nc.vector, nc.scalar]
            oeng = [nc.gpsimd, nc.sync]
            yer = [ye_d[e].rearrange("(m p) f -> p m f", p=128)
                   for e in range(E_LOC)]
            stages = [(e, m) for e in range(E_LOC) for m in range(MT)]
            aT_tiles = {}

            def trans(e, m, pool, tag):
                a = a_tiles[(e, m)]
                aT = apool.tile([128, IT, 128], dt.bfloat16, tag="aT",
                                name=f"aT_{e}_{m}")
                for k in range(IT):
                    pt = pool.tile([128, 128], dt.bfloat16, tag=tag,
                                   name=f"pt_{e}_{m}_{k}")
                    nc.tensor.transpose(
                        pt[:], a[:, k * 128:(k + 1) * 128], ident[:])
                    eng = ceng[k % 2]
                    eng.tensor_copy(aT[:, k, :], pt[:]) if eng is nc.vector \
                        else eng.copy(aT[:, k, :], pt[:])
                aT_tiles[(e, m)] = aT

            # yh shares the psg "sm" ring (no new PSUM pool => no allocation
            # barrier serializing mm2 behind acts-e1)
            ye_tiles = {}

            def mm2_piece(pool, tag, e, m, dq, push, split_copy=False):
                aT = aT_tiles[(e, m)]
                ye = ye_tiles[(e, m)]
                yh = pool.tile([128, 512], dt.float32, tag=tag,
                               name=f"yh_{e}_{m}_{dq}")
                for k in range(IT):
                    nc.tensor.matmul(
                        yh[:],
                        aT[:, k, :],
                        w2t[e][:, k, dq * 512:(dq + 1) * 512],
                        start=(k == 0), stop=(k == IT - 1))
                lo = dq * 512
                if split_copy:
                    nc.vector.tensor_copy(ye[:, lo:lo + 256], yh[:, 0:256])
                    nc.scalar.copy(ye[:, lo + 256:lo + 512], yh[:, 256:512])
                else:
                    eng = ceng[(dq + 1) % 2]
                    eng.tensor_copy(ye[:, lo:lo + 512], yh[:]) \
                        if eng is nc.vector else eng.copy(ye[:, lo:lo + 512], yh[:])
                for ring, plo, phi in push:
                    ring.dma_start(yer[e][:, m, plo:phi], ye[:, plo:phi])

            # T00 from the psg ring: it runs pre-barrier, bridging the
            # acts-e1 latency while the h pool drains.  Later stages'
            # transposes get a 3-deep ring in ps3 (alongside yh, both fit in
            # the released h-pool banks) so they never drip at copy pace
            # between mm2 runs.
            trans(*stages[0], psg, "sm")
            with tc.tile_pool(name="ps_3", bufs=3, space="PSUM") as ps3:
                for si, (e, m) in enumerate(stages):
                    ye_tiles[(e, m)] = youtp.tile([128, D], dt.bfloat16,
                                                  tag="ye", name=f"ye_{e}_{m}")
                    last = si == len(stages) - 1
                    for dq in range(DQ):
                        if last:
                            # final stage: per-piece pushes on gpsimd; final
                            # piece split in half (copy AND push) on the sync
                            # ring, whose output queue is kept empty so the
                            # last 128KB doesn't queue behind earlier pieces
                            if dq < 3:
                                push = [(nc.gpsimd, dq * 512, (dq + 1) * 512)]
                                mm2_piece(ps3, "yh", e, m, dq, push)
                            else:
                                push = [(nc.sync, 1536, 1792),
                                        (nc.sync, 1792, 2048)]
                                mm2_piece(ps3, "yh", e, m, dq, push,
                                          split_copy=True)
                        else:
                            push = []
                            if dq % 2 == 1:
                                push = [(nc.gpsimd,
                                         (dq - 1) * 512, (dq + 1) * 512)]
                            if si == 0 and dq < 2:
                                # first two pieces from the barrier-free psg
                                # ring: they run while the ps3 pool waits for
                                # acts-e1-m1 to release the h banks (~2.8us)
                                mm2_piece(psg, "sm", e, m, dq, push)
                            else:
                                mm2_piece(ps3, "yh", e, m, dq, push)
                    if si + 1 < len(stages):
                        trans(*stages[si + 1], ps3, "pt")

    nc.finalize()
    if split_waits:
        _split_excess_waits(nc)
    return nc


def kernel(hidden_states, topk_weights, topk_ids, w13_weight, w13_weight_scale,
           w2_weight, w2_weight_scale):
    from concourse.bass_utils import run_bass_kernel_spmd

    x = np.ascontiguousarray(hidden_states, dtype=np.float32)
    tw = np.asarray(topk_weights, dtype=np.float32)
    ti = np.asarray(topk_ids)

    # host routing: combine weights + per-expert token lists
    comb = np.zeros((T, E), np.float32)
    for k in range(TOPK):
        np.add.at(comb, (np.arange(T), ti[:, k]), tw[:, k])
    routed = comb > 0.0
    idx = [np.nonzero(routed[:, e])[0] for e in range(E)]
    counts = [len(ix) for ix in idx]
    cap = max(128, -(-max(counts) // 128) * 128)

    if cap not in _PROGRAM_CACHE:
        _PROGRAM_CACHE[cap] = _build_program(cap)
    nc = _PROGRAM_CACHE[cap]

    # weights: lossless host conversion (see module docstring)
    w13 = _dequant_mxfp4(np.asarray(w13_weight), np.asarray(w13_weight_scale))
    w2 = _dequant_mxfp4(np.asarray(w2_weight), np.asarray(w2_weight_scale))
    DT, IT, TT, MT = D // 128, I // 128, T // 128, cap // 128

    # x in partition-major column blocks: 256-col lead-in blocks, then
    # 512-col full blocks; shipped as fp16 (see _build_program docstring)
    NBF = len(BLK) - NBT
    xcb = (x.astype(np.float16)
           .reshape(TT, 128, DT, 128).transpose(1, 0, 2, 3))  # [128, c, j, f]
    xt = np.ascontiguousarray(
        xcb[:, :, :2 * NBT, :].reshape(128, TT, NBT, 256)
        .transpose(2, 0, 1, 3).reshape(NBT, 128, TT * 256))
    xb = np.ascontiguousarray(
        xcb[:, :, 2 * NBT:, :].reshape(128, TT, NBF, 512).transpose(2, 0, 1, 3)
        .reshape(NBF, 128, TT * 512))

    in_maps = []
    for core in range(N_CORES):
        m = {"xb": xb, "xt": xt}
        g = np.zeros((T, E_LOC * cap), FP8)
        cg = np.zeros((128, E_LOC * MT), np.float32)
        w13t = np.zeros((E_LOC, 128, DT * 2 * I), FP8)
        w2t = np.zeros((E_LOC, 128, IT * D), FP8)
        for le in range(E_LOC):
            e = core * E_LOC + le
            ix = idx[e]
            g[ix, le * cap + np.arange(len(ix))] = FP8(1.0)
            cvals = np.zeros(cap, np.float32)
            cvals[:len(ix)] = comb[ix, e]
            cg[:, le * MT:(le + 1) * MT] = cvals.reshape(MT, 128).T
            w13t[le] = (w13[e].T.astype(FP8).reshape(DT, 128, 2 * I)
                        .transpose(1, 0, 2).reshape(128, -1))
            w2t[le] = (w2[e].T.astype(FP8).reshape(IT, 128, D)
                       .transpose(1, 0, 2).reshape(128, -1))
        m["g"] = np.ascontiguousarray(
            g.reshape(TT, 128, E_LOC * cap).transpose(1, 0, 2).reshape(128, -1))
        m["combg"] = np.ascontiguousarray(cg)
        m["w13t"] = w13t
        m["w2t"] = w2t
        in_maps.append(m)

    res = run_bass_kernel_spmd(nc, in_maps, list(range(N_CORES)))

    out = np.zeros((T, D), np.float32)
    for core in range(N_CORES):
        ye = np.asarray(res.results[core]["ye"], dtype=np.float32)
        for le in range(E_LOC):
            e = core * E_LOC + le
            ix = idx[e]
            out[ix] += ye[le, :len(ix)]
    return out
